# revision 33
# baseline (speedup 1.0000x reference)
"""Bahdanau (additive) attention fused Trainium2 kernel.

Reference computation (per batch n):
    qc      = tanh(query[q,e] + context[v,e])            # [NQ, NV, NE]
    logits  = einsum('qve,e->qv', qc, w_logit) + b_logit
    probs   = softmax(logits / temp, axis=v)
    heads   = leaky_relu(probs @ memory, 0.01)           # [NQ, NE]
    out     = heads @ w_reduce.T + b_reduce              # [NQ, NE]

Sharding: 8 cores = 4 batches x 2 query-halves (data parallel, no
collectives).  Each core handles n = core//2, q-range = (core%2)*128..+128.

Per-core algorithm:
  - layout: e on partitions (2 tiles of 128), v on free dim.
  - DVE tensor_scalar_add broadcasts the query bias onto context rows
    (2x perf mode), producing pre-activation tiles [128 e, 512 v] per
    (q, e-tile); groups of 4 queries share one big ACT tanh op (FD 4096)
    to amortize the fixed per-op SBUF latency.  ACT tanh is the roofline
    (~109us/core at 1 elem/lane/cycle); DVE is the close second.
  - PE reduces over e using float32r matmuls (1 cycle/row vs 4 for fp32)
    with one-hot-expanded w_logit columns: lhsT [e=128, 32] has w on
    column q%32, so query q's logits land on PSUM row q%32 of block
    q//32.  f32r only supports col-group 0, hence four [32, 512] logits
    tiles.  tanh tiles are written as f32r by ACT (rounding producer).
  - per 32-query block, staged across following groups to spread load:
    A: ACT exp; B: PE transpose probs -> [v, q] (+1 DVE copy);
    C: PE matmul with [memory | ones] (ones column yields the softmax
       row-sum for free), DVE reciprocal + scaled leaky relu
       rc*max(x, .01x) = max(rc*.01*x, rc*x);
    D: PE transpose heads, matmul with w_reduce.T into out rows 32j
       (fp32 + tile_position col group), add b_reduce, store.

Host-side folds: w' = w_logit/temp (softmax temperature), b_logit dropped
(softmax shift invariance), w_reduce pre-transposed, memory gains a ones
column, b_reduce broadcast.  One-hot w tiles are built on device (memset
+ stride-33 diagonal copies).
"""

import sys

for _p in ("/opt/trn_rl_repo",):
    if _p not in sys.path:
        sys.path.insert(0, _p)

from contextlib import ExitStack

import numpy as np

import concourse.bass as bass
import concourse.tile as tile
from concourse import bacc, mybir
from concourse import bass_utils

F32 = mybir.dt.float32
F32R = mybir.dt.float32r

N, NQ, NV, NE = 4, 256, 512, 256
NCORES = 8
QH = NQ // 2          # queries per core
ET = NE // 128        # e tiles (partition dim)
VB = NV // 128        # v blocks of 128
G = 6                 # queries per ACT tanh group


def build_kernel_body(tc, ins, outs):
    nc = tc.nc
    ctxT_d, qT_d, mem_d, w_d, wrT_d, brb_d, ident_d = (
        ins["ctxT"], ins["qT"], ins["mem"], ins["w"], ins["wrT"],
        ins["brb"], ins["ident"],
    )
    out_d = outs["out"]

    with ExitStack() as ctx:
        consts = ctx.enter_context(tc.tile_pool(name="consts", bufs=1))
        spool = ctx.enter_context(tc.tile_pool(name="spool", bufs=3))
        tpool = ctx.enter_context(tc.tile_pool(name="tpool", bufs=3))
        small = ctx.enter_context(tc.tile_pool(name="small", bufs=1))
        pslog = ctx.enter_context(tc.tile_pool(name="pslog", bufs=1, space="PSUM"))
        pstr = ctx.enter_context(tc.tile_pool(name="pstr", bufs=2, space="PSUM"))
        psmm = ctx.enter_context(tc.tile_pool(name="psmm", bufs=1, space="PSUM"))

        # warm the ACT table (exp_and_others covers tanh+exp) during the
        # initial DMA wait so the first real tanh doesn't pay the load
        warm = consts.tile([128, 1], F32)
        nc.vector.memset(warm, 0.0)
        nc.scalar.activation(out=warm, in_=warm,
                             func=mybir.ActivationFunctionType.Tanh)

        # ---- constant loads: hot tensors split across all DGE queues ----
        ctxT_sb = consts.tile([128, ET, NV], F32)
        qT_sb = consts.tile([128, ET, QH], F32)
        H = NV // 2
        nc.sync.dma_start(out=ctxT_sb[:, 0, 0:H], in_=ctxT_d[0][:, 0:H])
        nc.scalar.dma_start(out=ctxT_sb[:, 0, H:NV], in_=ctxT_d[0][:, H:NV])
        nc.gpsimd.dma_start(out=qT_sb[:, 0, :], in_=qT_d[0])
        nc.sync.dma_start(out=ctxT_sb[:, 1, 0:H], in_=ctxT_d[1][:, 0:H])
        nc.scalar.dma_start(out=ctxT_sb[:, 1, H:NV], in_=ctxT_d[1][:, H:NV])
        nc.gpsimd.dma_start(out=qT_sb[:, 1, :], in_=qT_d[1])

        # one-hot expanded w_logit columns, built on device:
        # wpad[p, t, s, c] = w[t*128+p] * (c == s).  memset zeros, then
        # broadcast-copy the w column onto each 32x32 block diagonal
        # (free stride 33) with DVE.
        w_sb = consts.tile([128, ET], F32)
        nc.gpsimd.dma_start(out=w_sb, in_=w_d.rearrange("t p -> p t"))
        wpad_st = consts.tile([128, ET, 32, 32], F32)
        nc.vector.memset(wpad_st.rearrange("p t s c -> p (t s c)"), 0.0)
        for t in range(ET):
            blk = wpad_st[:, t, :, :]
            diag = bass.AP(tensor=blk.tensor, offset=blk.offset,
                           ap=[blk.ap[0], [33, 32]])
            wt = w_sb[:, t:t + 1]
            w_bcast = bass.AP(tensor=wt.tensor, offset=wt.offset,
                              ap=[wt.ap[0], [0, 32]])
            nc.vector.tensor_copy(diag, w_bcast)
        # rounded copy: f32r matmul operands must come from a rounding producer
        wpad_sb = consts.tile([128, ET, 32, 32], F32R)
        nc.vector.tensor_copy(
            wpad_sb.rearrange("p t s c -> p (t s c)"),
            wpad_st.rearrange("p t s c -> p (t s c)"),
        )
        mem_sb = consts.tile([128, VB, NE + 1], F32)
        nc.scalar.dma_start(out=mem_sb, in_=mem_d.rearrange("t p e -> p t e"))
        wrT_sb = consts.tile([128, ET, NE], F32)
        nc.gpsimd.dma_start(out=wrT_sb, in_=wrT_d.rearrange("t p o -> p t o"))
        brb_sb = consts.tile([128, NE], F32)
        nc.gpsimd.dma_start(out=brb_sb, in_=brb_d)
        ident_sb = consts.tile([128, 128], F32)
        nc.sync.dma_start(out=ident_sb, in_=ident_d)

        # f32r matmuls only support col-group 0, so logits live in four
        # [32, 512] PSUM tiles (q = 32j + s -> tile j, row s).
        lgs = [pslog.tile([32, NV], F32, name=f"lg{j}", tag=f"lg{j}")
               for j in range(4)]
        out_ps = psmm.tile([128, NE], F32, tag="ps3")
        outsb = small.tile([128, NE], F32)

        # Per-j epilogue, split into 4 stages that are emitted after
        # successive groups so the DVE/PE load spreads out instead of
        # starving ACT at block boundaries.
        st = {}

        def epi_a(j):
            ev = small.tile([32, NV], F32, name=f"ev{j}", tag=f"ev{j}")
            nc.scalar.activation(
                out=ev, in_=lgs[j], func=mybir.ActivationFunctionType.Exp)
            st[j] = {"ev": ev}

        def epi_b(j):
            ev = st[j]["ev"]
            # probs_j.T: [32, 512] -> [128, 4x32] via 4 PE transposes, 1 copy
            pT = small.tile([128, VB, 32], F32, name=f"pT{j}", tag=f"pT{j}")
            tps = pstr.tile([128, VB, 32], F32, name=f"tps{j}", tag="tps")
            for vb in range(VB):
                nc.tensor.transpose(
                    tps[:, vb, :], ev[:, 128 * vb:128 * (vb + 1)],
                    ident_sb[0:32, 0:32])
            nc.vector.tensor_copy(
                pT.rearrange("p a b -> p (a b)"),
                tps.rearrange("p a b -> p (a b)"))
            st[j]["pT"] = pT

        def epi_c(j):
            pT = st[j]["pT"]
            # heads_j = exp_j @ [memory | 1] : [32, 257]; col 256 = rowsum
            hps = psmm.tile([32, NE + 1], F32, name=f"hps{j}", tag="hps",
                            bufs=1)
            for vb in range(VB):
                nc.tensor.matmul(
                    out=hps, lhsT=pT[:, vb, :], rhs=mem_sb[:, vb, :],
                    start=(vb == 0), stop=(vb == VB - 1),
                )
            rc = small.tile([32, 1], F32, name=f"rc{j}", tag=f"rc{j}")
            nc.vector.reciprocal(rc, hps[:, NE:NE + 1])
            # leaky relu scaled by 1/rowsum: rc * max(x, 0.01x)
            rc01 = small.tile([32, 1], F32, name=f"rc01{j}", tag=f"rc01{j}")
            nc.vector.tensor_scalar_mul(rc01, rc, 0.01)
            h1 = small.tile([32, NE], F32, name=f"h1{j}", tag=f"h1{j}")
            nc.vector.tensor_scalar_mul(h1, hps[:, 0:NE], rc)
            hb = small.tile([32, NE], F32, name=f"hb{j}", tag=f"hb{j}")
            nc.vector.scalar_tensor_tensor(
                out=hb, in0=hps[:, 0:NE], scalar=rc01, in1=h1,
                op0=mybir.AluOpType.mult, op1=mybir.AluOpType.max,
            )
            st[j]["hb"] = hb

        def epi_d(j):
            hb = st[j]["hb"]
            # heads_j.T: [32, 256] -> [128, 2x32] via 2 PE transposes, 1 copy
            hT = small.tile([128, ET, 32], F32, name=f"hT{j}", tag=f"hT{j}")
            tps2 = pstr.tile([128, ET, 32], F32, name=f"tp2{j}", tag="tps")
            for eb in range(ET):
                nc.tensor.transpose(
                    tps2[:, eb, :], hb[:, 128 * eb:128 * (eb + 1)],
                    ident_sb[0:32, 0:32])
            nc.vector.tensor_copy(
                hT.rearrange("p a b -> p (a b)"),
                tps2.rearrange("p a b -> p (a b)"))
            # out rows 32j..32j+32 = heads_j @ w_reduce.T (fp32, col-group j)
            for eb in range(ET):
                nc.tensor.matmul(
                    out=out_ps[32 * j:32 * (j + 1), :],
                    lhsT=hT[:, eb, :],
                    rhs=wrT_sb[:, eb, :],
                    start=(eb == 0), stop=(eb == ET - 1),
                    tile_position=(0, 32 * j),
                )
            # add b_reduce (1/rowsum already folded in) and store this block
            nc.vector.tensor_add(
                outsb[32 * j:32 * (j + 1), :],
                out_ps[32 * j:32 * (j + 1), :],
                brb_sb[32 * j:32 * (j + 1), :],
            )
            dma_eng = [nc.sync, nc.scalar, nc.gpsimd, nc.sync][j]
            dma_eng.dma_start(out=out_d[32 * j:32 * (j + 1), :],
                              in_=outsb[32 * j:32 * (j + 1), :])

        # ---- main loop -------------------------------------------------
        # groups of G queries; the last block runs in groups of 2 so the
        # final PE drain before the tail epilogue is short.
        qlist = []
        q0 = 0
        while q0 < QH:
            rem = QH - q0
            cnt = G if rem > 8 else (4 if rem > 4 else 2)
            qlist.append((q0, cnt))
            q0 += cnt
        pending = []   # (due_group_idx, fn)
        for gi, (qs, cnt) in enumerate(qlist):
            sgrp = spool.tile([128, G, ET, NV], F32, name="sgrp", tag="sgrp")
            for t in range(ET):
                for i in range(cnt):
                    q = qs + i
                    nc.vector.tensor_scalar_add(
                        sgrp[:, i, t, :],
                        ctxT_sb[:, t, :],
                        qT_sb[:, t, q:q + 1],
                    )
            tgrp = tpool.tile([128, G, ET, NV], F32R, name="tgrp", tag="tgrp")
            if gi < 2:
                # startup: per-t halves so tanh starts after fewer adds
                for t in range(ET):
                    nc.scalar.activation(
                        out=tgrp[:, 0:cnt, t, :],
                        in_=sgrp[:, 0:cnt, t, :],
                        func=mybir.ActivationFunctionType.Tanh,
                    )
            else:
                nc.scalar.activation(
                    out=tgrp[:, 0:cnt, :, :].rearrange("p a t v -> p (a t v)"),
                    in_=sgrp[:, 0:cnt, :, :].rearrange("p a t v -> p (a t v)"),
                    func=mybir.ActivationFunctionType.Tanh,
                )
            for i in range(cnt):
                q = qs + i
                j, s = divmod(q, 32)
                for t in range(ET):
                    nc.tensor.matmul(
                        out=lgs[j],
                        lhsT=wpad_sb[:, t, s, :],
                        rhs=tgrp[:, i, t, :],
                        start=(s == 0 and t == 0),
                        stop=(s == 31 and t == ET - 1),
                    )
            # emit any due epilogue stages, schedule new ones
            qend = qs + cnt - 1
            for j in range(4):
                if not (qs <= 32 * j + 31 <= qend):
                    continue
                pending += [(gi, lambda j=j: epi_a(j)),
                            (gi + 1, lambda j=j: epi_b(j)),
                            (gi + 2, lambda j=j: epi_c(j)),
                            (gi + 3, lambda j=j: epi_d(j))]
            still = []
            for due, fn in pending:
                if due <= gi:
                    fn()
                else:
                    still.append((due, fn))
            pending = still
        for _, fn in sorted(pending, key=lambda x: x[0]):
            fn()


_CACHE = {}


def build_program():
    if "nc" in _CACHE:
        return _CACHE["nc"]
    nc = bacc.Bacc(
        "TRN2", target_bir_lowering=False, debug=False, num_devices=NCORES
    )
    ins = {
        "ctxT": nc.dram_tensor("ctxT", [ET, 128, NV], F32, kind="ExternalInput").ap(),
        "qT": nc.dram_tensor("qT", [ET, 128, QH], F32, kind="ExternalInput").ap(),
        "mem": nc.dram_tensor("mem", [VB, 128, NE + 1], F32, kind="ExternalInput").ap(),
        "w": nc.dram_tensor("w", [ET, 128], F32, kind="ExternalInput").ap(),
        "wrT": nc.dram_tensor("wrT", [ET, 128, NE], F32, kind="ExternalInput").ap(),
        "brb": nc.dram_tensor("brb", [128, NE], F32, kind="ExternalInput").ap(),
        "ident": nc.dram_tensor("ident", [128, 128], F32, kind="ExternalInput").ap(),
    }
    outs = {
        "out": nc.dram_tensor("out", [QH, NE], F32, kind="ExternalOutput").ap(),
    }
    with tile.TileContext(nc) as tc:
        build_kernel_body(tc, ins, outs)
    nc.compile()
    _CACHE["nc"] = nc
    return nc


def make_in_maps(query, context, memory, w_logit, b_logit, temp, w_reduce,
                 b_reduce):
    query = np.asarray(query, np.float32)
    context = np.asarray(context, np.float32)
    memory = np.asarray(memory, np.float32)
    w_logit = np.asarray(w_logit, np.float32)
    temp = np.asarray(temp, np.float32)
    w_reduce = np.asarray(w_reduce, np.float32)
    b_reduce = np.asarray(b_reduce, np.float32)

    w_scaled = (w_logit / temp).astype(np.float32)          # fold temperature
    w_in = np.ascontiguousarray(w_scaled.reshape(ET, 128))
    wrT = np.ascontiguousarray(w_reduce.T).reshape(ET, 128, NE)
    brb = np.ascontiguousarray(np.broadcast_to(b_reduce, (128, NE)))
    ident = np.eye(128, dtype=np.float32)

    in_maps = []
    for c in range(NCORES):
        n, h = divmod(c, 2)
        ctxT = np.ascontiguousarray(context[n].T).reshape(ET, 128, NV)
        qT = np.ascontiguousarray(
            query[n, h * QH:(h + 1) * QH].T).reshape(ET, 128, QH)
        mem = np.ascontiguousarray(
            np.concatenate([memory[n], np.ones((NV, 1), np.float32)], axis=1)
        ).reshape(VB, 128, NE + 1)
        in_maps.append({
            "ctxT": ctxT, "qT": qT, "mem": mem, "w": w_in,
            "wrT": wrT, "brb": brb, "ident": ident,
        })
    return in_maps


def gather_output(results):
    out = np.empty((N, NQ, NE), np.float32)
    for c in range(NCORES):
        n, h = divmod(c, 2)
        out[n, h * QH:(h + 1) * QH] = results[c]["out"]
    return out


def kernel(query, context, memory, w_logit, b_logit, temp, w_reduce,
           b_reduce, _trace=False):
    nc = build_program()
    in_maps = make_in_maps(query, context, memory, w_logit, b_logit, temp,
                           w_reduce, b_reduce)
    res = bass_utils.run_bass_kernel_spmd(
        nc, in_maps, core_ids=list(range(NCORES)), trace=_trace,
    )
    out = gather_output(res.results)
    if _trace:
        return out, res
    return out


if __name__ == "__main__":
    rng = np.random.default_rng(0)
    inputs = {
        "query": rng.standard_normal((N, NQ, NE), np.float32),
        "context": rng.standard_normal((N, NV, NE), np.float32),
        "memory": rng.standard_normal((N, NV, NE), np.float32),
        "w_logit": rng.standard_normal(NE, np.float32) / 16.0,
        "b_logit": np.float32(0.0),
        "temp": np.float32(1.0),
        "w_reduce": rng.standard_normal((NE, NE), np.float32) / 16.0,
        "b_reduce": np.zeros(NE, np.float32),
    }
    out = kernel(**inputs)
    print("out", out.shape, out.dtype, float(np.abs(out).mean()))


# revision 34
# speedup vs baseline: 1.0241x; 1.0241x over previous
"""Bahdanau (additive) attention fused Trainium2 kernel.

Reference computation (per batch n):
    qc      = tanh(query[q,e] + context[v,e])            # [NQ, NV, NE]
    logits  = einsum('qve,e->qv', qc, w_logit) + b_logit
    probs   = softmax(logits / temp, axis=v)
    heads   = leaky_relu(probs @ memory, 0.01)           # [NQ, NE]
    out     = heads @ w_reduce.T + b_reduce              # [NQ, NE]

Sharding: 8 cores = 4 batches x 2 query-halves (data parallel, no
collectives).  Each core handles n = core//2, q-range = (core%2)*128..+128.

Per-core algorithm:
  - layout: e on partitions (2 tiles of 128), v on free dim.
  - DVE tensor_scalar_add broadcasts the query bias onto context rows
    (2x perf mode), producing pre-activation tiles [128 e, 512 v] per
    (q, e-tile); groups of 4 queries share one big ACT tanh op (FD 4096)
    to amortize the fixed per-op SBUF latency.  ACT tanh is the roofline
    (~109us/core at 1 elem/lane/cycle); DVE is the close second.
  - PE reduces over e using float32r matmuls (1 cycle/row vs 4 for fp32)
    with one-hot-expanded w_logit columns: lhsT [e=128, 32] has w on
    column q%32, so query q's logits land on PSUM row q%32 of block
    q//32.  f32r only supports col-group 0, hence four [32, 512] logits
    tiles.  tanh tiles are written as f32r by ACT (rounding producer).
  - per 32-query block, staged across following groups to spread load:
    A: ACT exp; B: PE transpose probs -> [v, q] (+1 DVE copy);
    C: PE matmul with [memory | ones] (ones column yields the softmax
       row-sum for free), DVE reciprocal + scaled leaky relu
       rc*max(x, .01x) = max(rc*.01*x, rc*x);
    D: PE transpose heads, matmul with w_reduce.T into out rows 32j
       (fp32 + tile_position col group), add b_reduce, store.

Host-side folds: w' = w_logit/temp (softmax temperature), b_logit dropped
(softmax shift invariance), w_reduce pre-transposed, memory gains a ones
column, b_reduce broadcast.  One-hot w tiles are built on device (memset
+ stride-33 diagonal copies).
"""

import sys

for _p in ("/opt/trn_rl_repo",):
    if _p not in sys.path:
        sys.path.insert(0, _p)

from contextlib import ExitStack

import numpy as np

import concourse.bass as bass
import concourse.tile as tile
from concourse import bacc, mybir
from concourse import bass_utils

F32 = mybir.dt.float32
F32R = mybir.dt.float32r

N, NQ, NV, NE = 4, 256, 512, 256
NCORES = 8
QH = NQ // 2          # queries per core
ET = NE // 128        # e tiles (partition dim)
VB = NV // 128        # v blocks of 128
G = 4                 # queries per ACT tanh group


def build_kernel_body(tc, ins, outs):
    nc = tc.nc
    ctxT_d, qT_d, mem_d, w_d, wrT_d, brb_d, ident_d = (
        ins["ctxT"], ins["qT"], ins["mem"], ins["w"], ins["wrT"],
        ins["brb"], ins["ident"],
    )
    out_d = outs["out"]

    with ExitStack() as ctx:
        consts = ctx.enter_context(tc.tile_pool(name="consts", bufs=1))
        spool = ctx.enter_context(tc.tile_pool(name="spool", bufs=3))
        tpool = ctx.enter_context(tc.tile_pool(name="tpool", bufs=3))
        small = ctx.enter_context(tc.tile_pool(name="small", bufs=1))
        pslog = ctx.enter_context(tc.tile_pool(name="pslog", bufs=1, space="PSUM"))
        pstr = ctx.enter_context(tc.tile_pool(name="pstr", bufs=2, space="PSUM"))
        psmm = ctx.enter_context(tc.tile_pool(name="psmm", bufs=1, space="PSUM"))

        # warm the ACT table (exp_and_others covers tanh+exp) during the
        # initial DMA wait so the first real tanh doesn't pay the load
        warm = consts.tile([128, 1], F32)
        nc.vector.memset(warm, 0.0)
        nc.scalar.activation(out=warm, in_=warm,
                             func=mybir.ActivationFunctionType.Tanh)

        # ---- constant loads: hot tensors split across all DGE queues ----
        ctxT_sb = consts.tile([128, ET, NV], F32)
        qT_sb = consts.tile([128, ET, QH], F32)
        nc.gpsimd.dma_start(out=qT_sb[:, 0, :], in_=qT_d[0])
        nc.sync.dma_start(out=ctxT_sb[:, 0, 0:172], in_=ctxT_d[0][:, 0:172])
        nc.scalar.dma_start(out=ctxT_sb[:, 0, 172:344], in_=ctxT_d[0][:, 172:344])
        nc.gpsimd.dma_start(out=ctxT_sb[:, 0, 344:NV], in_=ctxT_d[0][:, 344:NV])
        nc.sync.dma_start(out=ctxT_sb[:, 1, 0:172], in_=ctxT_d[1][:, 0:172])
        nc.scalar.dma_start(out=ctxT_sb[:, 1, 172:344], in_=ctxT_d[1][:, 172:344])
        nc.gpsimd.dma_start(out=ctxT_sb[:, 1, 344:NV], in_=ctxT_d[1][:, 344:NV])
        nc.sync.dma_start(out=qT_sb[:, 1, :], in_=qT_d[1])

        # one-hot expanded w_logit columns, built on device:
        # wpad[p, t, s, c] = w[t*128+p] * (c == s).  memset zeros, then
        # broadcast-copy the w column onto each 32x32 block diagonal
        # (free stride 33) with DVE.
        w_sb = consts.tile([128, ET], F32)
        nc.gpsimd.dma_start(out=w_sb, in_=w_d.rearrange("t p -> p t"))
        wpad_st = consts.tile([128, ET, 32, 32], F32)
        nc.vector.memset(wpad_st.rearrange("p t s c -> p (t s c)"), 0.0)
        for t in range(ET):
            blk = wpad_st[:, t, :, :]
            diag = bass.AP(tensor=blk.tensor, offset=blk.offset,
                           ap=[blk.ap[0], [33, 32]])
            wt = w_sb[:, t:t + 1]
            w_bcast = bass.AP(tensor=wt.tensor, offset=wt.offset,
                              ap=[wt.ap[0], [0, 32]])
            nc.vector.tensor_copy(diag, w_bcast)
        # rounded copy: f32r matmul operands must come from a rounding producer
        wpad_sb = consts.tile([128, ET, 32, 32], F32R)
        nc.vector.tensor_copy(
            wpad_sb.rearrange("p t s c -> p (t s c)"),
            wpad_st.rearrange("p t s c -> p (t s c)"),
        )
        mem_sb = consts.tile([128, VB, NE + 1], F32)
        nc.scalar.dma_start(out=mem_sb, in_=mem_d.rearrange("t p e -> p t e"))
        wrT_sb = consts.tile([128, ET, NE], F32)
        nc.gpsimd.dma_start(out=wrT_sb, in_=wrT_d.rearrange("t p o -> p t o"))
        brb_sb = consts.tile([128, NE], F32)
        nc.gpsimd.dma_start(out=brb_sb, in_=brb_d)
        ident_sb = consts.tile([128, 128], F32)
        nc.sync.dma_start(out=ident_sb, in_=ident_d)

        # f32r matmuls only support col-group 0, so logits live in four
        # [32, 512] PSUM tiles (q = 32j + s -> tile j, row s).
        lgs = [pslog.tile([32, NV], F32, name=f"lg{j}", tag=f"lg{j}")
               for j in range(4)]
        out_ps = psmm.tile([128, NE], F32, tag="ps3")
        outsb = small.tile([128, NE], F32)

        # Per-j epilogue, split into 4 stages that are emitted after
        # successive groups so the DVE/PE load spreads out instead of
        # starving ACT at block boundaries.
        st = {}

        def epi_a(j):
            ev = small.tile([32, NV], F32, name=f"ev{j}", tag=f"ev{j}")
            nc.scalar.activation(
                out=ev, in_=lgs[j], func=mybir.ActivationFunctionType.Exp)
            st[j] = {"ev": ev}

        def epi_b(j):
            ev = st[j]["ev"]
            # probs_j.T: [32, 512] -> [128, 4x32] via 4 PE transposes, 1 copy
            pT = small.tile([128, VB, 32], F32, name=f"pT{j}", tag=f"pT{j}")
            tps = pstr.tile([128, VB, 32], F32, name=f"tps{j}", tag="tps")
            for vb in range(VB):
                nc.tensor.transpose(
                    tps[:, vb, :], ev[:, 128 * vb:128 * (vb + 1)],
                    ident_sb[0:32, 0:32])
            nc.vector.tensor_copy(
                pT.rearrange("p a b -> p (a b)"),
                tps.rearrange("p a b -> p (a b)"))
            st[j]["pT"] = pT

        def epi_c(j):
            pT = st[j]["pT"]
            # heads_j = exp_j @ [memory | 1] : [32, 257]; col 256 = rowsum
            hps = psmm.tile([32, NE + 1], F32, name=f"hps{j}", tag="hps",
                            bufs=1)
            for vb in range(VB):
                nc.tensor.matmul(
                    out=hps, lhsT=pT[:, vb, :], rhs=mem_sb[:, vb, :],
                    start=(vb == 0), stop=(vb == VB - 1),
                )
            rc = small.tile([32, 1], F32, name=f"rc{j}", tag=f"rc{j}")
            nc.vector.reciprocal(rc, hps[:, NE:NE + 1])
            # leaky relu scaled by 1/rowsum: rc * max(x, 0.01x)
            rc01 = small.tile([32, 1], F32, name=f"rc01{j}", tag=f"rc01{j}")
            nc.vector.tensor_scalar_mul(rc01, rc, 0.01)
            h1 = small.tile([32, NE], F32, name=f"h1{j}", tag=f"h1{j}")
            nc.vector.tensor_scalar_mul(h1, hps[:, 0:NE], rc)
            hb = small.tile([32, NE], F32, name=f"hb{j}", tag=f"hb{j}")
            nc.vector.scalar_tensor_tensor(
                out=hb, in0=hps[:, 0:NE], scalar=rc01, in1=h1,
                op0=mybir.AluOpType.mult, op1=mybir.AluOpType.max,
            )
            st[j]["hb"] = hb

        def epi_d(j):
            hb = st[j]["hb"]
            # heads_j.T: [32, 256] -> [128, 2x32] via 2 PE transposes, 1 copy
            hT = small.tile([128, ET, 32], F32, name=f"hT{j}", tag=f"hT{j}")
            tps2 = pstr.tile([128, ET, 32], F32, name=f"tp2{j}", tag="tps")
            for eb in range(ET):
                nc.tensor.transpose(
                    tps2[:, eb, :], hb[:, 128 * eb:128 * (eb + 1)],
                    ident_sb[0:32, 0:32])
            nc.vector.tensor_copy(
                hT.rearrange("p a b -> p (a b)"),
                tps2.rearrange("p a b -> p (a b)"))
            # out rows 32j..32j+32 = heads_j @ w_reduce.T (fp32, col-group j)
            for eb in range(ET):
                nc.tensor.matmul(
                    out=out_ps[32 * j:32 * (j + 1), :],
                    lhsT=hT[:, eb, :],
                    rhs=wrT_sb[:, eb, :],
                    start=(eb == 0), stop=(eb == ET - 1),
                    tile_position=(0, 32 * j),
                )
            # add b_reduce (1/rowsum already folded in) and store this block
            nc.vector.tensor_add(
                outsb[32 * j:32 * (j + 1), :],
                out_ps[32 * j:32 * (j + 1), :],
                brb_sb[32 * j:32 * (j + 1), :],
            )
            dma_eng = [nc.sync, nc.scalar, nc.gpsimd, nc.sync][j]
            dma_eng.dma_start(out=out_d[32 * j:32 * (j + 1), :],
                              in_=outsb[32 * j:32 * (j + 1), :])

        # ---- main loop -------------------------------------------------
        # groups of G queries; the last block runs in groups of 2 so the
        # final PE drain before the tail epilogue is short.
        qlist = []
        q0 = 0
        while q0 < QH:
            rem = QH - q0
            cnt = G if rem > 4 else 2
            qlist.append((q0, cnt))
            q0 += cnt
        pending = []   # (due_group_idx, fn)
        for gi, (qs, cnt) in enumerate(qlist):
            sgrp = spool.tile([128, G, ET, NV], F32, name="sgrp", tag="sgrp")
            for t in range(ET):
                for i in range(cnt):
                    q = qs + i
                    nc.vector.tensor_scalar_add(
                        sgrp[:, i, t, :],
                        ctxT_sb[:, t, :],
                        qT_sb[:, t, q:q + 1],
                    )
            tgrp = tpool.tile([128, G, ET, NV], F32R, name="tgrp", tag="tgrp")
            if gi < 2:
                # startup: per-t halves so tanh starts after fewer adds
                for t in range(ET):
                    nc.scalar.activation(
                        out=tgrp[:, 0:cnt, t, :],
                        in_=sgrp[:, 0:cnt, t, :],
                        func=mybir.ActivationFunctionType.Tanh,
                    )
            else:
                nc.scalar.activation(
                    out=tgrp[:, 0:cnt, :, :].rearrange("p a t v -> p (a t v)"),
                    in_=sgrp[:, 0:cnt, :, :].rearrange("p a t v -> p (a t v)"),
                    func=mybir.ActivationFunctionType.Tanh,
                )
            for i in range(cnt):
                q = qs + i
                j, s = divmod(q, 32)
                for t in range(ET):
                    nc.tensor.matmul(
                        out=lgs[j],
                        lhsT=wpad_sb[:, t, s, :],
                        rhs=tgrp[:, i, t, :],
                        start=(s == 0 and t == 0),
                        stop=(s == 31 and t == ET - 1),
                    )
            # emit any due epilogue stages, schedule new ones
            qend = qs + cnt - 1
            for j in range(4):
                if not (qs <= 32 * j + 31 <= qend):
                    continue
                pending += [(gi, lambda j=j: epi_a(j)),
                            (gi + 1, lambda j=j: epi_b(j)),
                            (gi + 2, lambda j=j: epi_c(j)),
                            (gi + 3, lambda j=j: epi_d(j))]
            still = []
            for due, fn in pending:
                if due <= gi:
                    fn()
                else:
                    still.append((due, fn))
            pending = still
        for _, fn in sorted(pending, key=lambda x: x[0]):
            fn()


_CACHE = {}


def build_program():
    if "nc" in _CACHE:
        return _CACHE["nc"]
    nc = bacc.Bacc(
        "TRN2", target_bir_lowering=False, debug=False, num_devices=NCORES
    )
    ins = {
        "ctxT": nc.dram_tensor("ctxT", [ET, 128, NV], F32, kind="ExternalInput").ap(),
        "qT": nc.dram_tensor("qT", [ET, 128, QH], F32, kind="ExternalInput").ap(),
        "mem": nc.dram_tensor("mem", [VB, 128, NE + 1], F32, kind="ExternalInput").ap(),
        "w": nc.dram_tensor("w", [ET, 128], F32, kind="ExternalInput").ap(),
        "wrT": nc.dram_tensor("wrT", [ET, 128, NE], F32, kind="ExternalInput").ap(),
        "brb": nc.dram_tensor("brb", [128, NE], F32, kind="ExternalInput").ap(),
        "ident": nc.dram_tensor("ident", [128, 128], F32, kind="ExternalInput").ap(),
    }
    outs = {
        "out": nc.dram_tensor("out", [QH, NE], F32, kind="ExternalOutput").ap(),
    }
    with tile.TileContext(nc) as tc:
        build_kernel_body(tc, ins, outs)
    nc.compile()
    _CACHE["nc"] = nc
    return nc


def make_in_maps(query, context, memory, w_logit, b_logit, temp, w_reduce,
                 b_reduce):
    query = np.asarray(query, np.float32)
    context = np.asarray(context, np.float32)
    memory = np.asarray(memory, np.float32)
    w_logit = np.asarray(w_logit, np.float32)
    temp = np.asarray(temp, np.float32)
    w_reduce = np.asarray(w_reduce, np.float32)
    b_reduce = np.asarray(b_reduce, np.float32)

    w_scaled = (w_logit / temp).astype(np.float32)          # fold temperature
    w_in = np.ascontiguousarray(w_scaled.reshape(ET, 128))
    wrT = np.ascontiguousarray(w_reduce.T).reshape(ET, 128, NE)
    brb = np.ascontiguousarray(np.broadcast_to(b_reduce, (128, NE)))
    ident = np.eye(128, dtype=np.float32)

    in_maps = []
    for c in range(NCORES):
        n, h = divmod(c, 2)
        ctxT = np.ascontiguousarray(context[n].T).reshape(ET, 128, NV)
        qT = np.ascontiguousarray(
            query[n, h * QH:(h + 1) * QH].T).reshape(ET, 128, QH)
        mem = np.ascontiguousarray(
            np.concatenate([memory[n], np.ones((NV, 1), np.float32)], axis=1)
        ).reshape(VB, 128, NE + 1)
        in_maps.append({
            "ctxT": ctxT, "qT": qT, "mem": mem, "w": w_in,
            "wrT": wrT, "brb": brb, "ident": ident,
        })
    return in_maps


def gather_output(results):
    out = np.empty((N, NQ, NE), np.float32)
    for c in range(NCORES):
        n, h = divmod(c, 2)
        out[n, h * QH:(h + 1) * QH] = results[c]["out"]
    return out


def kernel(query, context, memory, w_logit, b_logit, temp, w_reduce,
           b_reduce, _trace=False):
    nc = build_program()
    in_maps = make_in_maps(query, context, memory, w_logit, b_logit, temp,
                           w_reduce, b_reduce)
    res = bass_utils.run_bass_kernel_spmd(
        nc, in_maps, core_ids=list(range(NCORES)), trace=_trace,
    )
    out = gather_output(res.results)
    if _trace:
        return out, res
    return out


if __name__ == "__main__":
    rng = np.random.default_rng(0)
    inputs = {
        "query": rng.standard_normal((N, NQ, NE), np.float32),
        "context": rng.standard_normal((N, NV, NE), np.float32),
        "memory": rng.standard_normal((N, NV, NE), np.float32),
        "w_logit": rng.standard_normal(NE, np.float32) / 16.0,
        "b_logit": np.float32(0.0),
        "temp": np.float32(1.0),
        "w_reduce": rng.standard_normal((NE, NE), np.float32) / 16.0,
        "b_reduce": np.zeros(NE, np.float32),
    }
    out = kernel(**inputs)
    print("out", out.shape, out.dtype, float(np.abs(out).mean()))


# revision 35
# speedup vs baseline: 1.0397x; 1.0153x over previous
"""Bahdanau (additive) attention fused Trainium2 kernel.

Reference computation (per batch n):
    qc      = tanh(query[q,e] + context[v,e])            # [NQ, NV, NE]
    logits  = einsum('qve,e->qv', qc, w_logit) + b_logit
    probs   = softmax(logits / temp, axis=v)
    heads   = leaky_relu(probs @ memory, 0.01)           # [NQ, NE]
    out     = heads @ w_reduce.T + b_reduce              # [NQ, NE]

Sharding: 8 cores = 4 batches x 2 query-halves (data parallel, no
collectives).  Each core handles n = core//2, q-range = (core%2)*128..+128.

Per-core algorithm:
  - layout: e on partitions (2 tiles of 128), v on free dim.
  - DVE tensor_scalar_add broadcasts the query bias onto context rows
    (2x perf mode), producing pre-activation tiles [128 e, 512 v] per
    (q, e-tile); groups of 4 queries share one big ACT tanh op (FD 4096)
    to amortize the fixed per-op SBUF latency.  ACT tanh is the roofline
    (~109us/core at 1 elem/lane/cycle); DVE is the close second.
  - PE reduces over e using float32r matmuls (1 cycle/row vs 4 for fp32)
    with one-hot-expanded w_logit columns: lhsT [e=128, 32] has w on
    column q%32, so query q's logits land on PSUM row q%32 of block
    q//32.  f32r only supports col-group 0, hence four [32, 512] logits
    tiles.  tanh tiles are written as f32r by ACT (rounding producer).
  - per 32-query block, staged across following groups to spread load:
    A: ACT exp; B: PE transpose probs -> [v, q] (+1 DVE copy);
    C: PE matmul with [memory | ones] (ones column yields the softmax
       row-sum for free), DVE reciprocal + scaled leaky relu
       rc*max(x, .01x) = max(rc*.01*x, rc*x);
    D: PE transpose heads, matmul with w_reduce.T into out rows 32j
       (fp32 + tile_position col group), add b_reduce, store.

Host-side folds: w' = w_logit/temp (softmax temperature), b_logit dropped
(softmax shift invariance), w_reduce pre-transposed, memory gains a ones
column, b_reduce broadcast.  One-hot w tiles are built on device (memset
+ stride-33 diagonal copies).
"""

import sys

for _p in ("/opt/trn_rl_repo",):
    if _p not in sys.path:
        sys.path.insert(0, _p)

from contextlib import ExitStack

import numpy as np

import concourse.bass as bass
import concourse.tile as tile
from concourse import bacc, mybir
from concourse import bass_utils

F32 = mybir.dt.float32
F32R = mybir.dt.float32r

N, NQ, NV, NE = 4, 256, 512, 256
NCORES = 8
QH = NQ // 2          # queries per core
ET = NE // 128        # e tiles (partition dim)
VB = NV // 128        # v blocks of 128
G = 4                 # queries per ACT tanh group


def build_kernel_body(tc, ins, outs):
    nc = tc.nc
    ctxT_d, qT_d, mem_d, w_d, wrT_d, brb_d, ident_d = (
        ins["ctxT"], ins["qT"], ins["mem"], ins["w"], ins["wrT"],
        ins["brb"], ins["ident"],
    )
    out_d = outs["out"]

    with ExitStack() as ctx:
        consts = ctx.enter_context(tc.tile_pool(name="consts", bufs=1))
        spool = ctx.enter_context(tc.tile_pool(name="spool", bufs=3))
        tpool = ctx.enter_context(tc.tile_pool(name="tpool", bufs=3))
        small = ctx.enter_context(tc.tile_pool(name="small", bufs=1))
        pslog = ctx.enter_context(tc.tile_pool(name="pslog", bufs=1, space="PSUM"))
        pstr = ctx.enter_context(tc.tile_pool(name="pstr", bufs=2, space="PSUM"))
        psmm = ctx.enter_context(tc.tile_pool(name="psmm", bufs=1, space="PSUM"))

        # warm the ACT table (exp_and_others covers tanh+exp) during the
        # initial DMA wait so the first real tanh doesn't pay the load
        warm = consts.tile([128, 1], F32)
        nc.vector.memset(warm, 0.0)
        nc.scalar.activation(out=warm, in_=warm,
                             func=mybir.ActivationFunctionType.Tanh)

        # ---- constant loads: hot tensors split across all DGE queues ----
        ctxT_sb = consts.tile([128, ET, NV], F32)
        qT_sb = consts.tile([128, ET, QH], F32)
        H = NV // 2
        nc.sync.dma_start(out=ctxT_sb[:, 0, 0:H], in_=ctxT_d[0][:, 0:H])
        nc.scalar.dma_start(out=ctxT_sb[:, 0, H:NV], in_=ctxT_d[0][:, H:NV])
        nc.gpsimd.dma_start(out=qT_sb[:, 0, :], in_=qT_d[0])
        nc.sync.dma_start(out=ctxT_sb[:, 1, 0:H], in_=ctxT_d[1][:, 0:H])
        nc.scalar.dma_start(out=ctxT_sb[:, 1, H:NV], in_=ctxT_d[1][:, H:NV])
        nc.gpsimd.dma_start(out=qT_sb[:, 1, :], in_=qT_d[1])

        # one-hot expanded w_logit columns, built on device:
        # wpad[p, t, s, c] = w[t*128+p] * (c == s).  memset zeros, then
        # broadcast-copy the w column onto each 32x32 block diagonal
        # (free stride 33) with DVE.
        w_sb = consts.tile([128, ET], F32)
        nc.gpsimd.dma_start(out=w_sb, in_=w_d.rearrange("t p -> p t"))
        wpad_st = consts.tile([128, ET, 32, 32], F32)
        nc.vector.memset(wpad_st.rearrange("p t s c -> p (t s c)"), 0.0)
        for t in range(ET):
            blk = wpad_st[:, t, :, :]
            diag = bass.AP(tensor=blk.tensor, offset=blk.offset,
                           ap=[blk.ap[0], [33, 32]])
            wt = w_sb[:, t:t + 1]
            w_bcast = bass.AP(tensor=wt.tensor, offset=wt.offset,
                              ap=[wt.ap[0], [0, 32]])
            nc.vector.tensor_copy(diag, w_bcast)
        # rounded copy: f32r matmul operands must come from a rounding producer
        wpad_sb = consts.tile([128, ET, 32, 32], F32R)
        nc.vector.tensor_copy(
            wpad_sb.rearrange("p t s c -> p (t s c)"),
            wpad_st.rearrange("p t s c -> p (t s c)"),
        )
        mem_sb = consts.tile([128, VB, NE + 1], F32)
        nc.scalar.dma_start(out=mem_sb, in_=mem_d.rearrange("t p e -> p t e"))
        wrT_sb = consts.tile([128, ET, NE], F32)
        nc.gpsimd.dma_start(out=wrT_sb, in_=wrT_d.rearrange("t p o -> p t o"))
        brb_sb = consts.tile([128, NE], F32)
        nc.gpsimd.dma_start(out=brb_sb, in_=brb_d)
        ident_sb = consts.tile([128, 128], F32)
        nc.sync.dma_start(out=ident_sb, in_=ident_d)

        # f32r matmuls only support col-group 0, so logits live in four
        # [32, 512] PSUM tiles (q = 32j + s -> tile j, row s).
        lgs = [pslog.tile([32, NV], F32, name=f"lg{j}", tag=f"lg{j}")
               for j in range(4)]
        out_ps = psmm.tile([128, NE], F32, tag="ps3")
        outsb = small.tile([128, NE], F32)

        # Per-j epilogue, split into 4 stages that are emitted after
        # successive groups so the DVE/PE load spreads out instead of
        # starving ACT at block boundaries.
        st = {}

        def epi_a(j):
            ev = small.tile([32, NV], F32, name=f"ev{j}", tag=f"ev{j}")
            nc.scalar.activation(
                out=ev, in_=lgs[j], func=mybir.ActivationFunctionType.Exp)
            st[j] = {"ev": ev}

        def epi_b(j):
            ev = st[j]["ev"]
            # probs_j.T: [32, 512] -> [128, 4x32] via 4 PE transposes, 1 copy
            pT = small.tile([128, VB, 32], F32, name=f"pT{j}", tag=f"pT{j}")
            tps = pstr.tile([128, VB, 32], F32, name=f"tps{j}", tag="tps")
            for vb in range(VB):
                nc.tensor.transpose(
                    tps[:, vb, :], ev[:, 128 * vb:128 * (vb + 1)],
                    ident_sb[0:32, 0:32])
            nc.vector.tensor_copy(
                pT.rearrange("p a b -> p (a b)"),
                tps.rearrange("p a b -> p (a b)"))
            st[j]["pT"] = pT

        def epi_c(j):
            pT = st[j]["pT"]
            # heads_j = exp_j @ [memory | 1] : [32, 257]; col 256 = rowsum
            hps = psmm.tile([32, NE + 1], F32, name=f"hps{j}", tag="hps",
                            bufs=1)
            for vb in range(VB):
                nc.tensor.matmul(
                    out=hps, lhsT=pT[:, vb, :], rhs=mem_sb[:, vb, :],
                    start=(vb == 0), stop=(vb == VB - 1),
                )
            rc = small.tile([32, 1], F32, name=f"rc{j}", tag=f"rc{j}")
            nc.vector.reciprocal(rc, hps[:, NE:NE + 1])
            # leaky relu scaled by 1/rowsum: rc * max(x, 0.01x)
            rc01 = small.tile([32, 1], F32, name=f"rc01{j}", tag=f"rc01{j}")
            nc.vector.tensor_scalar_mul(rc01, rc, 0.01)
            h1 = small.tile([32, NE], F32, name=f"h1{j}", tag=f"h1{j}")
            nc.vector.tensor_scalar_mul(h1, hps[:, 0:NE], rc)
            hb = small.tile([32, NE], F32, name=f"hb{j}", tag=f"hb{j}")
            nc.vector.scalar_tensor_tensor(
                out=hb, in0=hps[:, 0:NE], scalar=rc01, in1=h1,
                op0=mybir.AluOpType.mult, op1=mybir.AluOpType.max,
            )
            st[j]["hb"] = hb

        def epi_d(j):
            hb = st[j]["hb"]
            # heads_j.T: [32, 256] -> [128, 2x32] via 2 PE transposes, 1 copy
            hT = small.tile([128, ET, 32], F32, name=f"hT{j}", tag=f"hT{j}")
            tps2 = pstr.tile([128, ET, 32], F32, name=f"tp2{j}", tag="tps")
            for eb in range(ET):
                nc.tensor.transpose(
                    tps2[:, eb, :], hb[:, 128 * eb:128 * (eb + 1)],
                    ident_sb[0:32, 0:32])
            nc.vector.tensor_copy(
                hT.rearrange("p a b -> p (a b)"),
                tps2.rearrange("p a b -> p (a b)"))
            # out rows 32j..32j+32 = heads_j @ w_reduce.T (fp32, col-group j)
            for eb in range(ET):
                nc.tensor.matmul(
                    out=out_ps[32 * j:32 * (j + 1), :],
                    lhsT=hT[:, eb, :],
                    rhs=wrT_sb[:, eb, :],
                    start=(eb == 0), stop=(eb == ET - 1),
                    tile_position=(0, 32 * j),
                )
            # add b_reduce (1/rowsum already folded in) and store this block
            nc.vector.tensor_add(
                outsb[32 * j:32 * (j + 1), :],
                out_ps[32 * j:32 * (j + 1), :],
                brb_sb[32 * j:32 * (j + 1), :],
            )
            dma_eng = [nc.sync, nc.scalar, nc.gpsimd, nc.sync][j]
            dma_eng.dma_start(out=out_d[32 * j:32 * (j + 1), :],
                              in_=outsb[32 * j:32 * (j + 1), :])

        # ---- main loop -------------------------------------------------
        # groups of G queries; the last block runs in groups of 2 so the
        # final PE drain before the tail epilogue is short.
        qlist = []
        q0 = 0
        while q0 < QH:
            rem = QH - q0
            cnt = G if rem > 4 else 2
            qlist.append((q0, cnt))
            q0 += cnt
        pending = []   # (due_group_idx, fn)
        for gi, (qs, cnt) in enumerate(qlist):
            sgrp = spool.tile([128, G, ET, NV], F32, name="sgrp", tag="sgrp")
            for t in range(ET):
                for i in range(cnt):
                    q = qs + i
                    nc.vector.tensor_scalar_add(
                        sgrp[:, i, t, :],
                        ctxT_sb[:, t, :],
                        qT_sb[:, t, q:q + 1],
                    )
            tgrp = tpool.tile([128, G, ET, NV], F32R, name="tgrp", tag="tgrp")
            if gi < 2:
                # startup: per-t halves so tanh starts after fewer adds
                for t in range(ET):
                    nc.scalar.activation(
                        out=tgrp[:, 0:cnt, t, :],
                        in_=sgrp[:, 0:cnt, t, :],
                        func=mybir.ActivationFunctionType.Tanh,
                    )
            else:
                nc.scalar.activation(
                    out=tgrp[:, 0:cnt, :, :].rearrange("p a t v -> p (a t v)"),
                    in_=sgrp[:, 0:cnt, :, :].rearrange("p a t v -> p (a t v)"),
                    func=mybir.ActivationFunctionType.Tanh,
                )
            for i in range(cnt):
                q = qs + i
                j, s = divmod(q, 32)
                for t in range(ET):
                    nc.tensor.matmul(
                        out=lgs[j],
                        lhsT=wpad_sb[:, t, s, :],
                        rhs=tgrp[:, i, t, :],
                        start=(s == 0 and t == 0),
                        stop=(s == 31 and t == ET - 1),
                    )
            # emit any due epilogue stages, schedule new ones
            qend = qs + cnt - 1
            for j in range(4):
                if not (qs <= 32 * j + 31 <= qend):
                    continue
                pending += [(gi, lambda j=j: epi_a(j)),
                            (gi + 1, lambda j=j: epi_b(j)),
                            (gi + 2, lambda j=j: epi_c(j)),
                            (gi + 3, lambda j=j: epi_d(j))]
            still = []
            for due, fn in pending:
                if due <= gi:
                    fn()
                else:
                    still.append((due, fn))
            pending = still
        for _, fn in sorted(pending, key=lambda x: x[0]):
            fn()


_CACHE = {}


def build_program():
    if "nc" in _CACHE:
        return _CACHE["nc"]
    nc = bacc.Bacc(
        "TRN2", target_bir_lowering=False, debug=False, num_devices=NCORES
    )
    ins = {
        "ctxT": nc.dram_tensor("ctxT", [ET, 128, NV], F32, kind="ExternalInput").ap(),
        "qT": nc.dram_tensor("qT", [ET, 128, QH], F32, kind="ExternalInput").ap(),
        "mem": nc.dram_tensor("mem", [VB, 128, NE + 1], F32, kind="ExternalInput").ap(),
        "w": nc.dram_tensor("w", [ET, 128], F32, kind="ExternalInput").ap(),
        "wrT": nc.dram_tensor("wrT", [ET, 128, NE], F32, kind="ExternalInput").ap(),
        "brb": nc.dram_tensor("brb", [128, NE], F32, kind="ExternalInput").ap(),
        "ident": nc.dram_tensor("ident", [128, 128], F32, kind="ExternalInput").ap(),
    }
    outs = {
        "out": nc.dram_tensor("out", [QH, NE], F32, kind="ExternalOutput").ap(),
    }
    with tile.TileContext(nc) as tc:
        build_kernel_body(tc, ins, outs)
    nc.compile()
    _CACHE["nc"] = nc
    return nc


def make_in_maps(query, context, memory, w_logit, b_logit, temp, w_reduce,
                 b_reduce):
    query = np.asarray(query, np.float32)
    context = np.asarray(context, np.float32)
    memory = np.asarray(memory, np.float32)
    w_logit = np.asarray(w_logit, np.float32)
    temp = np.asarray(temp, np.float32)
    w_reduce = np.asarray(w_reduce, np.float32)
    b_reduce = np.asarray(b_reduce, np.float32)

    w_scaled = (w_logit / temp).astype(np.float32)          # fold temperature
    w_in = np.ascontiguousarray(w_scaled.reshape(ET, 128))
    wrT = np.ascontiguousarray(w_reduce.T).reshape(ET, 128, NE)
    brb = np.ascontiguousarray(np.broadcast_to(b_reduce, (128, NE)))
    ident = np.eye(128, dtype=np.float32)

    in_maps = []
    for c in range(NCORES):
        n, h = divmod(c, 2)
        ctxT = np.ascontiguousarray(context[n].T).reshape(ET, 128, NV)
        qT = np.ascontiguousarray(
            query[n, h * QH:(h + 1) * QH].T).reshape(ET, 128, QH)
        mem = np.ascontiguousarray(
            np.concatenate([memory[n], np.ones((NV, 1), np.float32)], axis=1)
        ).reshape(VB, 128, NE + 1)
        in_maps.append({
            "ctxT": ctxT, "qT": qT, "mem": mem, "w": w_in,
            "wrT": wrT, "brb": brb, "ident": ident,
        })
    return in_maps


def gather_output(results):
    out = np.empty((N, NQ, NE), np.float32)
    for c in range(NCORES):
        n, h = divmod(c, 2)
        out[n, h * QH:(h + 1) * QH] = results[c]["out"]
    return out


def kernel(query, context, memory, w_logit, b_logit, temp, w_reduce,
           b_reduce, _trace=False):
    nc = build_program()
    in_maps = make_in_maps(query, context, memory, w_logit, b_logit, temp,
                           w_reduce, b_reduce)
    res = bass_utils.run_bass_kernel_spmd(
        nc, in_maps, core_ids=list(range(NCORES)), trace=_trace,
    )
    out = gather_output(res.results)
    if _trace:
        return out, res
    return out


if __name__ == "__main__":
    rng = np.random.default_rng(0)
    inputs = {
        "query": rng.standard_normal((N, NQ, NE), np.float32),
        "context": rng.standard_normal((N, NV, NE), np.float32),
        "memory": rng.standard_normal((N, NV, NE), np.float32),
        "w_logit": rng.standard_normal(NE, np.float32) / 16.0,
        "b_logit": np.float32(0.0),
        "temp": np.float32(1.0),
        "w_reduce": rng.standard_normal((NE, NE), np.float32) / 16.0,
        "b_reduce": np.zeros(NE, np.float32),
    }
    out = kernel(**inputs)
    print("out", out.shape, out.dtype, float(np.abs(out).mean()))
